# revision 11
# baseline (speedup 1.0000x reference)
"""GNN message-passing gather kernel for Trainium2 (8 NeuronCores).

reference semantics:
    msg_src = node_states[:, edge_src, :]       # [B, E, D]
    msg_tgt = node_states[:, edge_tgt, :]       # [B, E, D]
    out     = concat([msg_src, msg_tgt], -1)    # [B, E, 2D]

Strategy: shard edges across the 8 cores (20000 each); every core holds a
full fp16 replica of node_states in local HBM (fp16 round-trip rel err is
2^-11 ~ 5e-4, inside the 2e-2 gate, and halves HBM traffic vs fp32).

The SWDGE gather path is Q7-descriptor-emission-bound (~3.4 ns/descriptor;
measured: random, sorted, and 4KB-hot-set index distributions all gather at
the same speed), so the layout is chosen to minimize descriptor count, not
to improve HBM locality: the node table is packed node-major on the host
(row i = concat over b of node_states[b, i, :], 2 KiB/row) so ONE
descriptor per edge fetches all 4 batches. That cuts descriptors 4x; the
gather becomes HBM-bound.

Per (tile of 1024 edges, src/tgt list) one dma_gather pulls 1024 x 2KiB
rows HBM->SBUF and one HWDGE store pushes the tile to an edge-major
[EC_PAD, B, D] slab (host untransposes to [B, EC, D] during assembly, the
same copy it already performs). Edge indices are pre-permuted on the host
so gather row c*128+p carries edge 8p+c: each SBUF partition holds 8
consecutive edge rows, making every store descriptor a contiguous 16KiB
block.
"""

import numpy as np

import concourse.bass as bass
import concourse.tile as tile
from concourse import bacc, mybir
from concourse.bass_utils import run_bass_kernel_spmd

B, N, D, E = 4, 10000, 256, 160000
BD = B * D                  # packed row: 1024 fp16 elems = 2 KiB
NCORES = 8
EC = E // NCORES            # 20000 edges per core
TILE_EDGES = 1024           # rows per dma_gather call (2048 trips a HW ring limit)
CHUNKS = TILE_EDGES // 128  # free-dim chunks in one gather tile
NT = 20                     # tiles per list
EC_PAD = NT * TILE_EDGES    # 20480 (padded with index 0; sliced off on host)
IDX_COLS = EC_PAD // 16     # wrapped int16 index columns


def build_program(n=N, bd=BD, ec_pad=EC_PAD, tile_edges=TILE_EDGES,
                  num_devices=NCORES, debug=False, gather_bufs=4, loop_n=1,
                  gather_mode="gather", store=True, single_packet=True,
                  body_repeat=1):
    """Build + compile the per-core Bass program (identical on all cores).

    loop_n>1 wraps the whole body in a hardware For_i loop (same output
    regions every iteration) - bench-only knob for slope-based exec timing.
    gather_mode "seq" and store=False are bench-only ablations (wrong output).
    """
    nt = ec_pad // tile_edges
    chunks = tile_edges // 128
    idx_cols = ec_pad // 16
    cols_per_tile = tile_edges // 16

    nc = bacc.Bacc("TRN2", target_bir_lowering=False, debug=debug,
                   enable_asserts=debug, num_devices=num_devices)

    node = nc.dram_tensor("node_packed", [n, bd], mybir.dt.float16,
                          kind="ExternalInput")
    idx_src = nc.dram_tensor("idx_src", [128, idx_cols], mybir.dt.int16,
                             kind="ExternalInput")
    idx_tgt = nc.dram_tensor("idx_tgt", [128, idx_cols], mybir.dt.int16,
                             kind="ExternalInput")
    outs = {
        "src": nc.dram_tensor("out_src", [ec_pad, bd], mybir.dt.float16,
                              kind="ExternalOutput"),
        "tgt": nc.dram_tensor("out_tgt", [ec_pad, bd], mybir.dt.float16,
                              kind="ExternalOutput"),
    }

    with tile.TileContext(nc) as tc:
        with tc.tile_pool(name="idx", bufs=1) as idx_pool, \
             tc.tile_pool(name="gather", bufs=gather_bufs) as gpool:
            idx_sb = {}
            for name, dram in (("src", idx_src), ("tgt", idx_tgt)):
                t = idx_pool.tile([128, idx_cols], mybir.dt.int16, tag=name)
                nc.sync.dma_start(out=t[:], in_=dram.ap())
                idx_sb[name] = t

            def body():
              for _ in range(body_repeat):
                for j in range(nt):
                    for name in ("src", "tgt"):
                        gt = gpool.tile([128, chunks, bd], mybir.dt.float16,
                                        tag="gt")
                        if gather_mode == "gather":
                            nc.gpsimd.dma_gather(
                                gt[:],
                                node.ap(),
                                idx_sb[name][:, j * cols_per_tile:(j + 1) * cols_per_tile],
                                tile_edges,
                                tile_edges,
                                bd,
                                single_packet=single_packet,
                            )
                        elif gather_mode == "seq":
                            rows = (j * tile_edges) % (n - tile_edges)
                            seq_ap = bass.AP(
                                node, rows * bd,
                                [[chunks * bd, 128], [bd, chunks], [1, bd]])
                            nc.sync.dma_start(out=gt[:], in_=seq_ap)
                        # gather row c*128+p = edge 8p+c (host permuted), so
                        # partition p holds edges [8p, 8p+8): one contiguous
                        # 16KiB block per partition.
                        if store:
                            dram_ap = bass.AP(
                                outs[name],
                                j * tile_edges * bd,
                                [[chunks * bd, 128], [bd, chunks], [1, bd]],
                            )
                            nc.sync.dma_start(out=dram_ap, in_=gt[:])

            if loop_n == 1:
                body()
            else:
                with tc.For_i(0, loop_n, 1):
                    body()

    nc.compile()
    return nc


def _prep_idx(idx, tile_edges=TILE_EDGES, chunks=CHUNKS):
    """[EC_PAD] int -> [128, EC_PAD//16] int16 SWDGE-wrapped index layout,
    with a per-tile permutation so gather row c*128+p carries edge 8p+c.

    Gather-call-local entry g lives at partition g%16, column g//16 of the
    call's index window; replicated to all 128 partitions.
    """
    nt = idx.shape[0] // tile_edges
    # permute: entry g = c*128+p must hold original edge 8p+c
    a = idx.astype(np.int16).reshape(nt, 128, chunks)
    a = a.transpose(0, 2, 1).reshape(nt * tile_edges)
    # wrap: entry g -> partition g%16, column g//16 (within the call window)
    a = a.reshape(nt, tile_edges // 16, 16)
    a = a.transpose(2, 0, 1).reshape(16, nt * (tile_edges // 16))
    return np.ascontiguousarray(np.tile(a, (8, 1)))


_PROGRAM = None
LAST_RESULTS = None


def _get_program():
    global _PROGRAM
    if _PROGRAM is None:
        _PROGRAM = build_program()
    return _PROGRAM


def pack_node_table(node_states):
    """[B, N, D] fp32 -> [N, B*D] fp16 node-major packed table."""
    ns = np.asarray(node_states).astype(np.float16)
    return np.ascontiguousarray(ns.transpose(1, 0, 2).reshape(N, BD))


def run_programs(nc, node_states, edge_src, edge_tgt):
    """Shard inputs, run the given program on all 8 cores, return results."""
    packed = pack_node_table(node_states)
    es = np.asarray(edge_src).astype(np.int64, copy=False)
    et = np.asarray(edge_tgt).astype(np.int64, copy=False)

    pad = np.zeros(EC_PAD - EC, np.int64)
    in_maps = []
    for k in range(NCORES):
        sl = slice(k * EC, (k + 1) * EC)
        in_maps.append({
            "node_packed": packed,
            "idx_src": _prep_idx(np.concatenate([es[sl], pad])),
            "idx_tgt": _prep_idx(np.concatenate([et[sl], pad])),
        })
    return run_bass_kernel_spmd(nc, in_maps, core_ids=list(range(NCORES)))


def kernel(node_states, edge_src, edge_tgt):
    nc = _get_program()
    res = run_programs(nc, node_states, edge_src, edge_tgt)
    global LAST_RESULTS
    LAST_RESULTS = res

    full = np.empty((B, E, 2 * D), np.float32)
    for k in range(NCORES):
        sl = slice(k * EC, (k + 1) * EC)
        # out_* rows are edge-major [EC_PAD, B, D]; untranspose to [B, EC, D]
        src = res.results[k]["out_src"][:EC].reshape(EC, B, D)
        tgt = res.results[k]["out_tgt"][:EC].reshape(EC, B, D)
        full[:, sl, :D] = src.transpose(1, 0, 2)
        full[:, sl, D:] = tgt.transpose(1, 0, 2)
    return full
